# revision 1
# baseline (speedup 1.0000x reference)
"""Multi-head attention (B=2, S=2048, D=768, H=12) on 8 Trainium2 cores.

Sharding: core c -> batch b = c // 4, head-group g = c % 4 (3 heads of 12).
Each core gets its batch's activations pre-transposed on the host (x^T
[768, 2048] fp32 — a pure layout permutation, part of shard prep) plus its
head-group's weight shards.  Each core computes Q/K/V projections for its
head group, attention, and a partial output (its head rows of Wo).  The
host sums the 4 partials per batch and adds bo.

Device kernel (per core):
  - x^T loaded with a casting DMA (fp32 -> bf16, 8KB/partition contiguous
    descriptors) straight into the projection rhs layout.
  - Q^T, K^T per head as [64, 2048] bf16 tiles (lhsT = W chunks); V natural
    [2048, 3*65] bf16 with a ones column per head (the softmax denominator
    rides the PV matmul).
  - scores computed transposed: S^T[k, q] = K Q^T on PE; exp on ScalarE
    (scale = 1/sqrt(64), PSUM->SBUF bf16); PV matmul V_aug^T @ P^T
    accumulates O^T[65, q] in PSUM, row 64 = denominator.  q is processed
    in halves of 1024, heads 0/1 interleaved in the k loop, and both
    scores emitted before either PV so the in-order PE queue never blocks
    on the ACT exp (keeps PE streaming and HAM at K=8/8).
  - normalize O^T with approx-reciprocal + partition-broadcast off the
    critical path; Wo row-shard matmuls (interleaved into the solo head's
    k loop) produce the partial [2048, 768] fp32 output.
"""

import sys

for _p in ("/opt/trn_rl_repo",):
    if _p not in sys.path:
        sys.path.append(_p)

import numpy as np

B = 2
S = 2048
D = 768
H = 12
DK = 64
HG = 3            # heads per core
HD = HG * DK      # 192
P = 128
NS = S // P       # 16 s-tiles
ND = D // P       # 6 d-chunks
NB = S // 512     # 4 s-blocks
QH = 1024         # q half

_CACHE = {}


def _build_nc(use_bias_qkv):
    import concourse.bacc as bacc
    import concourse.tile as tile
    from concourse import mybir
    from contextlib import ExitStack

    BF = mybir.dt.bfloat16
    F32 = mybir.dt.float32
    EXP = mybir.ActivationFunctionType.Exp

    nc = bacc.Bacc("TRN2", target_bir_lowering=False, debug=False)

    xqT = nc.dram_tensor("xqT", [D, S], F32, kind="ExternalInput").ap()
    xkT = nc.dram_tensor("xkT", [D, S], F32, kind="ExternalInput").ap()
    xvT = nc.dram_tensor("xvT", [D, S], F32, kind="ExternalInput").ap()
    wq = nc.dram_tensor("wq", [D, HD], F32, kind="ExternalInput").ap()
    wk = nc.dram_tensor("wk", [D, HD], F32, kind="ExternalInput").ap()
    wv = nc.dram_tensor("wv", [D, HD], F32, kind="ExternalInput").ap()
    wo = nc.dram_tensor("wo", [HD, D], F32, kind="ExternalInput").ap()
    bqkv = nc.dram_tensor("bqkv", [3, HD], F32, kind="ExternalInput").ap()
    y = nc.dram_tensor("y", [S, D], F32, kind="ExternalOutput").ap()

    with tile.TileContext(nc) as tc, ExitStack() as ctx:
        wpool = ctx.enter_context(tc.tile_pool(name="weights", bufs=1))
        apool = ctx.enter_context(tc.tile_pool(name="acts", bufs=1))

        QTa = apool.tile([P, S], BF, tag="qta")    # heads 0,1 stacked on partitions
        QTb = apool.tile([DK, S], BF, tag="qtb")   # head 2
        KTa = apool.tile([P, S], BF, tag="kta")
        KTb = apool.tile([DK, S], BF, tag="ktb")
        QT = [QTa[0:DK, :], QTa[DK:P, :], QTb[:, :]]
        KT = [KTa[0:DK, :], KTa[DK:P, :], KTb[:, :]]
        V = apool.tile([P, NS, 3 * 65], BF, tag="v")
        OC1 = apool.tile([P, S], BF, tag="oc1")    # heads 0,1 of O^T (normalized)
        OC2 = apool.tile([DK, S], BF, tag="oc2")   # head 2

        # ================= phase 1: load x^T + projections =================
        # inputs loaded v, k, q as per-d-chunk casting DMAs so projections
        # chase the loads; Q/K projections run d-outer over s-block pairs so
        # the stationary weight is reused across streams (few LDWEIGHTS).
        with tc.tile_pool(name="xt", bufs=2) as xt_pool, \
             tc.tile_pool(name="mm_ps", bufs=2, space="PSUM") as mm_pool, \
             tc.tile_pool(name="qka_ps", bufs=1, space="PSUM") as qka_pool, \
             tc.tile_pool(name="qkb_ps", bufs=1, space="PSUM") as qkb_pool:

            # x^T loads: one whole-tile casting DMA per (input, d-chunk) so
            # downstream matmuls chase individual chunk arrivals
            xtc = {}
            for name, xT in (("wv", xvT), ("wk", xkT), ("wq", xqT)):
                for dc in range(ND):
                    t = xt_pool.tile([P, S], BF, tag=f"xt{dc}", name=f"xt_{name}{dc}")
                    nc.gpsimd.dma_start(out=t, in_=xT[dc * P : (dc + 1) * P, :])
                    xtc[(name, dc)] = t

            # weights (HWDGE queue, parallel with the gpsimd loads)
            w_bf = {}
            for name, w in (("wv", wv), ("wk", wk), ("wq", wq)):
                wf = wpool.tile([P, ND, HD], F32, tag=f"{name}_f32", name=f"{name}_f32")
                nc.sync.dma_start(out=wf, in_=w.rearrange("(nd p) h -> p nd h", p=P))
                wb = wpool.tile([P, ND, HD], BF, tag=f"{name}_bf", name=f"{name}_bf")
                nc.any.tensor_copy(out=wb, in_=wf)
                w_bf[name] = wb
            wo_f1 = wpool.tile([P, D], F32, tag="wo_f1")
            nc.sync.dma_start(out=wo_f1, in_=wo[0:P, :])
            wo_f2 = wpool.tile([DK, D], F32, tag="wo_f2")
            nc.sync.dma_start(out=wo_f2, in_=wo[P:HD, :])
            wo_b1 = wpool.tile([P, D], BF, tag="wo_b1")
            nc.any.tensor_copy(out=wo_b1, in_=wo_f1)
            wo_b2 = wpool.tile([DK, D], BF, tag="wo_b2")
            nc.any.tensor_copy(out=wo_b2, in_=wo_f2)
            nc.vector.memset(V[:, :, 64 : 3 * 65 : 65], 1.0)

            bias_a = {}
            bias_b = {}
            if use_bias_qkv:
                for i, name in enumerate(("wq", "wk", "wv")):
                    ba = wpool.tile([P, 1], F32, tag=f"ba_{name}", name=f"ba_{name}")
                    nc.sync.dma_start(out=ba, in_=bqkv[i, 0:P].rearrange("p -> p 1"))
                    bb = wpool.tile([DK, 1], F32, tag=f"bb_{name}", name=f"bb_{name}")
                    nc.sync.dma_start(out=bb, in_=bqkv[i, P:HD].rearrange("p -> p 1"))
                    bias_a[name] = ba
                    bias_b[name] = bb

            def v_proj():
                # V natural: [128(s), 192] per s-tile = x @ Wv.  Emitted
                # between the K and Q projections: it has no DMA dependency
                # left by then, so it keeps the PE dense (HAM warm) while
                # the xq chunks stream in.
                wb = w_bf["wv"]
                for st in range(NS):
                    psV = mm_pool.tile([P, HD], F32, tag="mm", name="psV")
                    for d in range(ND):
                        nc.tensor.matmul(
                            psV, xtc[("wv", d)][:, st * P : (st + 1) * P], wb[:, d, :],
                            start=(d == 0), stop=(d == ND - 1),
                        )
                    for h in range(HG):
                        nc.any.tensor_copy(
                            out=V[:, st, h * 65 : h * 65 + 64],
                            in_=psV[:, h * DK : (h + 1) * DK],
                        )

            # K^T / Q^T: d-outer over s-block pairs (stationary W reused,
            # chunks consumed as their DMAs land)
            for name, dstA, dstB in (("wk", KTa, KTb), ("wq", QTa, QTb)):
                if name == "wq":
                    v_proj()
                wb = w_bf[name]
                for sbp in range(NB // 2):
                    ssl = slice(sbp * QH, (sbp + 1) * QH)
                    psA = qka_pool.tile([P, QH], F32, tag="qka", name="psA")
                    psB = qkb_pool.tile([DK, QH], F32, tag="qkb", name="psB")
                    for d in range(ND):
                        xt_d = xtc[(name, d)]
                        for half in range(2):
                            hsl = slice(half * 512, (half + 1) * 512)
                            xsl = slice(sbp * QH + half * 512, sbp * QH + (half + 1) * 512)
                            nc.tensor.matmul(
                                psA[:, hsl], wb[:, d, 0:P], xt_d[:, xsl],
                                start=(d == 0), stop=(d == ND - 1),
                            )
                        for half in range(2):
                            hsl = slice(half * 512, (half + 1) * 512)
                            xsl = slice(sbp * QH + half * 512, sbp * QH + (half + 1) * 512)
                            nc.tensor.matmul(
                                psB[:, hsl], wb[:, d, P:HD], xt_d[:, xsl],
                                start=(d == 0), stop=(d == ND - 1),
                            )
                    if use_bias_qkv:
                        nc.vector.tensor_scalar_add(dstA[:, ssl], psA, bias_a[name])
                        nc.vector.tensor_scalar_add(dstB[:, ssl], psB, bias_b[name])
                    else:
                        nc.any.tensor_copy(out=dstA[:, ssl], in_=psA)
                        nc.any.tensor_copy(out=dstB[:, ssl], in_=psB)

        # ============ phase 2+3: attention (+ interleaved Wo) ============
        with tc.tile_pool(name="s_ps", bufs=2, space="PSUM") as s_pool, \
             tc.tile_pool(name="ot_ps", bufs=2, space="PSUM") as ot_pool, \
             tc.tile_pool(name="pt", bufs=3) as pt_pool, \
             tc.tile_pool(name="nrm", bufs=2) as nrm_pool, \
             tc.tile_pool(name="y_sb", bufs=2) as ysb_pool:

            def scores(h, kt, qh):
                tp = (0, 0) if h == 0 else ((64, 0) if h == 1 else None)
                s_ps = s_pool.tile([P, QH], F32, tag="s", name="s_ps")
                for n in range(QH // 512):
                    q0 = qh * QH + n * 512
                    nc.tensor.matmul(
                        s_ps[:, n * 512 : (n + 1) * 512],
                        KT[h][:, kt * P : (kt + 1) * P],
                        QT[h][:, q0 : q0 + 512],
                        start=True, stop=True, tile_position=tp,
                    )
                pt = pt_pool.tile([P, QH], BF, tag="pt", name="pt")
                nc.scalar.activation(pt, s_ps, EXP, bias=0.0, scale=0.125)
                return pt

            def pv(h, kt, ot, pt):
                for n in range(QH // 512):
                    nc.tensor.matmul(
                        ot[:, n * 512 : (n + 1) * 512],
                        V[:, kt, h * 65 : (h + 1) * 65],
                        pt[:, n * 512 : (n + 1) * 512],
                        start=(kt == 0), stop=(kt == NS - 1),
                    )

            def normalize(h, qh, ot):
                osb = nrm_pool.tile([DK, QH], F32, tag="osb", name="osb")
                nc.vector.tensor_copy(out=osb, in_=ot[0:DK, :])
                den = nrm_pool.tile([1, QH], F32, tag="den", name="den")
                nc.vector.tensor_copy(out=den, in_=ot[64:65, :])
                recip = nrm_pool.tile([1, QH], F32, tag="recip", name="recip")
                nc.vector.reciprocal_approx_fast(recip, den)
                rbc = nrm_pool.tile([DK, QH], F32, tag="rbc", name="rbc")
                nc.gpsimd.partition_broadcast(rbc, recip)
                sl = slice(qh * QH, (qh + 1) * QH)
                dst = OC1[0:DK, sl] if h == 0 else (OC1[DK:P, sl] if h == 1 else OC2[:, sl])
                nc.vector.tensor_mul(dst, osb, rbc)

            y_r = y.rearrange("(n p) m -> n p m", p=P)

            def wo_tile(st):
                y_ps = ot_pool.tile([P, D], F32, tag="ot", name="y_ps")
                sl = slice(st * P, (st + 1) * P)
                for n0, nn in ((0, 512), (512, 256)):
                    nc.tensor.matmul(
                        y_ps[:, n0 : n0 + nn], OC1[:, sl], wo_b1[:, n0 : n0 + nn],
                        start=True, stop=False,
                    )
                    nc.tensor.matmul(
                        y_ps[:, n0 : n0 + nn], OC2[:, sl], wo_b2[:, n0 : n0 + nn],
                        start=False, stop=True,
                    )
                y_sb = ysb_pool.tile([P, D], F32, tag="ysb", name="y_sb")
                nc.vector.tensor_copy(out=y_sb, in_=y_ps)
                nc.sync.dma_start(out=y_r[st], in_=y_sb)

            for qh in range(S // QH):
                # paired heads 0,1: both scores before both PVs so the
                # in-order PE queue never stalls behind an exp wait
                ot01 = [
                    ot_pool.tile([65, QH], F32, tag="ot", name=f"ot{h}_{qh}")
                    for h in range(2)
                ]
                def scores_pair(kt):
                    sps = [s_pool.tile([P, QH], F32, tag="s", name=f"s_ps{h}") for h in range(2)]
                    for n in range(QH // 512):
                        q0 = qh * QH + n * 512
                        for h in range(2):
                            nc.tensor.matmul(
                                sps[h][:, n * 512 : (n + 1) * 512],
                                KT[h][:, kt * P : (kt + 1) * P],
                                QT[h][:, q0 : q0 + 512],
                                start=True, stop=True,
                                tile_position=(64 * h, 0),
                            )
                    out = []
                    for h in range(2):
                        pt = pt_pool.tile([P, QH], BF, tag="pt", name="pt")
                        nc.scalar.activation(pt, sps[h], EXP, bias=0.0, scale=0.125)
                        out.append(pt)
                    return out

                pts = scores_pair(0)
                for kt in range(NS):
                    nxt = scores_pair(kt + 1) if kt < NS - 1 else [None, None]
                    for h in range(2):
                        pv(h, kt, ot01[h], pts[h])
                    pts = nxt

                for h in range(2):
                    normalize(h, qh, ot01[h])
                # solo head 2, software-pipelined, with the previous
                # q-half's Wo tiles interleaved as PE filler
                ot2 = ot_pool.tile([65, QH], F32, tag="ot", name=f"ot2_{qh}")
                pt2 = scores(2, 0, qh)
                for kt in range(NS):
                    nxt = scores(2, kt + 1, qh) if kt < NS - 1 else None
                    pv(2, kt, ot2, pt2)
                    pt2 = nxt
                    if qh > 0 and kt % 2 == 1:
                        wo_tile((qh - 1) * (NS // 2) + kt // 2)
                normalize(2, qh, ot2)
            for st in range(NS // 2, NS):
                wo_tile(st)

    nc.compile()
    return nc


def kernel(query, key, value, Wq, bq, Wk, bk, Wv, bv, Wo, bo, **_ignored):
    from concourse.bass_utils import run_bass_kernel_spmd

    query = np.asarray(query, dtype=np.float32)
    key = np.asarray(key, dtype=np.float32)
    value = np.asarray(value, dtype=np.float32)
    Wq = np.asarray(Wq, dtype=np.float32)
    Wk = np.asarray(Wk, dtype=np.float32)
    Wv = np.asarray(Wv, dtype=np.float32)
    Wo = np.asarray(Wo, dtype=np.float32)
    bq = np.asarray(bq, dtype=np.float32)
    bk = np.asarray(bk, dtype=np.float32)
    bv = np.asarray(bv, dtype=np.float32)
    bo = np.asarray(bo, dtype=np.float32)

    use_bias_qkv = bool(np.any(bq) or np.any(bk) or np.any(bv))
    if "nc" not in _CACHE or _CACHE.get("bias") != use_bias_qkv:
        _CACHE["nc"] = _build_nc(use_bias_qkv)
        _CACHE["bias"] = use_bias_qkv
    nc = _CACHE["nc"]

    xT = {b: {} for b in range(B)}
    for b in range(B):
        xT[b]["q"] = np.ascontiguousarray(query[b].T)
        xT[b]["k"] = np.ascontiguousarray(key[b].T)
        xT[b]["v"] = np.ascontiguousarray(value[b].T)

    in_maps = []
    for c in range(8):
        b, g = divmod(c, 4)
        hs = slice(g * HD, (g + 1) * HD)
        in_maps.append({
            "xqT": xT[b]["q"],
            "xkT": xT[b]["k"],
            "xvT": xT[b]["v"],
            "wq": np.ascontiguousarray(Wq[:, hs]),
            "wk": np.ascontiguousarray(Wk[:, hs]),
            "wv": np.ascontiguousarray(Wv[:, hs]),
            "wo": np.ascontiguousarray(Wo[hs, :]),
            "bqkv": np.ascontiguousarray(
                np.stack([bq[hs], bk[hs], bv[hs]]).astype(np.float32)
            ),
        })

    res = run_bass_kernel_spmd(nc, in_maps, core_ids=list(range(8)), **_CACHE.get("run_kwargs", {}))
    _CACHE["last_result"] = res

    out = np.empty((B, S, D), dtype=np.float32)
    for b in range(B):
        acc = res.results[4 * b]["y"].astype(np.float32).copy()
        for g in range(1, 4):
            acc += res.results[4 * b + g]["y"]
        out[b] = acc + bo[None, :]
    return out



# revision 5
# speedup vs baseline: 1.5827x; 1.5827x over previous
"""Multi-head attention (B=2, S=2048, D=768, H=12) on 8 Trainium2 cores.

Sharding: core c -> batch b = c // 4, head-group g = c % 4 (3 heads of 12).
Host prep: x^T per batch pre-transposed AND cast to bf16 (halves the x DMA
vs fp32+casting-DMA); weight shards cast to bf16.  Each core projects
Q/K/V for its 3 heads, runs attention, emits its Wo row-shard partial as
bf16; the host sums 4 partials per batch in fp32 and adds bo.

Device kernel (per core):
  - Q^T/K^T stored zero-PADDED to 128 contraction rows ([128, 3, S] tiles,
    rows 64-127 = 0) so every scores matmul is a full 128x128-array
    instruction: the 64-row (half-array / HAM k=4) config measured ~2x
    slower sustained on HW (activity throttle), and padding costs no extra
    PE cycles (row count = rhs free size).
  - All matmul outputs are <=512 fp32 columns (one PSUM bank; 1024-col out
    is an ISA violation, probed).
  - Attention runs as 6 phases (qh-major: (h0,h1,h2) x qh0 then qh1), each
    16 kt steps of: scores 2mm -> exp (ScalarE, [128,1024] tiles) -> PV
    2mm accumulating [65,1024] (ones column in V_aug rides the softmax
    denominator).  A global 2-step software pipeline (scores of step i+2
    emitted before PV of step i) keeps the in-order PE queue from ever
    waiting on the ACT exp, across phase boundaries too.
  - Wo tiles for the first q-half are emitted right after (h2,qh0)'s
    normalize, shortening the serial tail to normalize + 8 wo tiles.
"""

import sys

for _p in ("/opt/trn_rl_repo",):
    if _p not in sys.path:
        sys.path.append(_p)

import numpy as np

B = 2
S = 2048
D = 768
H = 12
DK = 64
HG = 3            # heads per core
HD = HG * DK      # 192
P = 128
NS = S // P       # 16 k-tiles
ND = D // P       # 6 d-chunks
QH = 1024         # q half

_CACHE = {}


def _build_nc(use_bias_qkv):
    import concourse.bacc as bacc
    import concourse.tile as tile
    from concourse import mybir
    from contextlib import ExitStack

    BF = mybir.dt.bfloat16
    F32 = mybir.dt.float32
    EXP = mybir.ActivationFunctionType.Exp

    nc = bacc.Bacc("TRN2", target_bir_lowering=False, debug=False)

    xqT = nc.dram_tensor("xqT", [D, S], BF, kind="ExternalInput").ap()
    xkT = nc.dram_tensor("xkT", [D, S], BF, kind="ExternalInput").ap()
    xvT = nc.dram_tensor("xvT", [D, S], BF, kind="ExternalInput").ap()
    wq = nc.dram_tensor("wq", [D, HD], BF, kind="ExternalInput").ap()
    wk = nc.dram_tensor("wk", [D, HD], BF, kind="ExternalInput").ap()
    wv = nc.dram_tensor("wv", [D, HD], BF, kind="ExternalInput").ap()
    wo = nc.dram_tensor("wo", [HD, D], BF, kind="ExternalInput").ap()
    bqkv = nc.dram_tensor("bqkv", [3, HD], F32, kind="ExternalInput").ap()
    y = nc.dram_tensor("y", [S, D], BF, kind="ExternalOutput").ap()

    with tile.TileContext(nc) as tc, ExitStack() as ctx:
        wpool = ctx.enter_context(tc.tile_pool(name="weights", bufs=1))
        apool = ctx.enter_context(tc.tile_pool(name="acts", bufs=1))

        # zero-padded transposed activations: [:, h, :] = head h, rows 64+ = 0
        KT = apool.tile([P, HG, S], BF, tag="kt")
        QT = apool.tile([P, HG, S], BF, tag="qt")
        V = apool.tile([P, NS, 3 * 65], BF, tag="v")
        OC1 = apool.tile([P, S], BF, tag="oc1")    # heads 0,1 of O^T (normalized)
        OC2 = apool.tile([DK, S], BF, tag="oc2")   # head 2

        # x chunk tiles (bf16 straight from HBM), all resident
        xt_pool = ctx.enter_context(tc.tile_pool(name="xt", bufs=1))
        xtc = {}
        for name, xT in (("wk", xkT), ("wq", xqT), ("wv", xvT)):
            for dc in range(ND):
                t = xt_pool.tile([P, S], BF, tag=f"xt_{name}{dc}", name=f"xt_{name}{dc}")
                nc.gpsimd.dma_start(out=t, in_=xT[dc * P : (dc + 1) * P, :])
                xtc[(name, dc)] = t

        # weights (bf16 on host, no device casts), HWDGE queue
        w_bf = {}
        for name, w in (("wk", wk), ("wq", wq), ("wv", wv)):
            wb = wpool.tile([P, ND, HD], BF, tag=f"{name}_bf", name=f"{name}_bf")
            nc.sync.dma_start(out=wb, in_=w.rearrange("(nd p) h -> p nd h", p=P))
            w_bf[name] = wb
        wo_b1 = wpool.tile([P, D], BF, tag="wo_b1")
        nc.sync.dma_start(out=wo_b1, in_=wo[0:P, :])
        wo_b2 = wpool.tile([DK, D], BF, tag="wo_b2")
        nc.sync.dma_start(out=wo_b2, in_=wo[P:HD, :])

        bias_a = {}
        bias_b = {}
        bias_vrow = None
        if use_bias_qkv:
            for i, name in enumerate(("wq", "wk", "wv")):
                ba = wpool.tile([P, 1], F32, tag=f"ba_{name}", name=f"ba_{name}")
                nc.sync.dma_start(out=ba, in_=bqkv[i, 0:P].rearrange("p -> p 1"))
                bb = wpool.tile([DK, 1], F32, tag=f"bb_{name}", name=f"bb_{name}")
                nc.sync.dma_start(out=bb, in_=bqkv[i, P:HD].rearrange("p -> p 1"))
                bias_a[name] = ba
                bias_b[name] = bb
            # V bias varies along the free dim of psV [s, 192]: broadcast the
            # bias row across all 128 partitions once
            bvr = wpool.tile([1, HD], F32, tag="bv_row")
            nc.sync.dma_start(out=bvr, in_=bqkv[2, :].rearrange("h -> 1 h"))
            bias_vrow = wpool.tile([P, HD], F32, tag="bv_bcast")
            nc.gpsimd.partition_broadcast(bias_vrow, bvr)

        # padding zeros + V ones columns (off the PE path; after DMA triggers)
        nc.gpsimd.memset(KT[DK:P, :, :], 0.0)
        nc.vector.memset(QT[DK:P, :, :], 0.0)
        nc.vector.memset(V[:, :, 64 : 3 * 65 : 65], 1.0)

        # ================= phase 1: projections =================
        with tc.tile_pool(name="ppa", bufs=2, space="PSUM") as ppa_pool, \
             tc.tile_pool(name="ppb", bufs=1, space="PSUM") as ppb_pool, \
             tc.tile_pool(name="psv", bufs=2, space="PSUM") as psv_pool:

            def qk_proj(name, dst):
                wb = w_bf[name]
                for sbp in range(2):
                    sl = slice(sbp * QH, (sbp + 1) * QH)
                    psA = ppa_pool.tile([P, QH], F32, tag="ppa", name=f"psA_{name}{sbp}")
                    psB = ppb_pool.tile([DK, QH], F32, tag="ppb", name=f"psB_{name}{sbp}")
                    for d in range(ND):
                        xt_d = xtc[(name, d)]
                        for half in range(2):
                            hsl = slice(half * 512, (half + 1) * 512)
                            xsl = slice(sbp * QH + half * 512, sbp * QH + (half + 1) * 512)
                            nc.tensor.matmul(
                                psA[:, hsl], wb[:, d, 0:P], xt_d[:, xsl],
                                start=(d == 0), stop=(d == ND - 1),
                            )
                    for d in range(ND):
                        xt_d = xtc[(name, d)]
                        for half in range(2):
                            hsl = slice(half * 512, (half + 1) * 512)
                            xsl = slice(sbp * QH + half * 512, sbp * QH + (half + 1) * 512)
                            nc.tensor.matmul(
                                psB[:, hsl], wb[:, d, P:HD], xt_d[:, xsl],
                                start=(d == 0), stop=(d == ND - 1),
                            )
                    if use_bias_qkv:
                        ba, bb = bias_a[name], bias_b[name]
                        nc.vector.tensor_scalar_add(dst[0:DK, 0, sl], psA[0:DK, :], ba[0:DK])
                        nc.vector.tensor_scalar_add(dst[0:DK, 1, sl], psA[DK:P, :], ba[DK:P])
                        nc.vector.tensor_scalar_add(dst[0:DK, 2, sl], psB, bb)
                    else:
                        nc.vector.tensor_copy(out=dst[0:DK, 0, sl], in_=psA[0:DK, :])
                        nc.vector.tensor_copy(out=dst[0:DK, 1, sl], in_=psA[DK:P, :])
                        nc.vector.tensor_copy(out=dst[0:DK, 2, sl], in_=psB)

            qk_proj("wk", KT)
            qk_proj("wq", QT)

            wb = w_bf["wv"]
            for st in range(NS):
                psV = psv_pool.tile([P, HD], F32, tag="psv", name=f"psV{st}")
                for d in range(ND):
                    nc.tensor.matmul(
                        psV, xtc[("wv", d)][:, st * P : (st + 1) * P], wb[:, d, :],
                        start=(d == 0), stop=(d == ND - 1),
                    )
                for h in range(HG):
                    if use_bias_qkv:
                        nc.vector.tensor_add(
                            V[:, st, h * 65 : h * 65 + 64],
                            psV[:, h * DK : (h + 1) * DK],
                            bias_vrow[:, h * DK : (h + 1) * DK],
                        )
                    else:
                        nc.vector.tensor_copy(
                            out=V[:, st, h * 65 : h * 65 + 64],
                            in_=psV[:, h * DK : (h + 1) * DK],
                        )

        # ============ phase 2: attention (+ wo) ============
        with tc.tile_pool(name="s_ps", bufs=2, space="PSUM") as s_pool, \
             tc.tile_pool(name="ot_ps", bufs=2, space="PSUM") as ot_pool, \
             tc.tile_pool(name="pt", bufs=4) as pt_pool, \
             tc.tile_pool(name="nrm", bufs=2) as nrm_pool, \
             tc.tile_pool(name="y_sb", bufs=2) as ysb_pool:

            # qh-major so wo for q-half 0 can run before the final phase
            phases = [(h, 0) for h in range(HG)] + [(h, 1) for h in range(HG)]
            ots = {}
            pts = {}

            def s_exp(i):
                h, qh = phases[i // NS]
                kt = i % NS
                s_ps = s_pool.tile([P, QH], F32, tag="s", name=f"s{i}")
                for n in range(2):
                    q0 = qh * QH + n * 512
                    nc.tensor.matmul(
                        s_ps[:, n * 512 : (n + 1) * 512],
                        KT[:, h, kt * P : (kt + 1) * P],
                        QT[:, h, q0 : q0 + 512],
                        start=True, stop=True,
                    )
                pt = pt_pool.tile([P, QH], BF, tag="pt", name=f"pt{i}")
                nc.scalar.activation(pt, s_ps, EXP, bias=0.0, scale=0.125)
                pts[i] = pt

            def pv(i):
                h, qh = phases[i // NS]
                kt = i % NS
                if kt == 0:
                    ots[(h, qh)] = ot_pool.tile([65, QH], F32, tag="ot", name=f"ot{h}_{qh}")
                ot = ots[(h, qh)]
                pt = pts.pop(i)
                for n in range(2):
                    nc.tensor.matmul(
                        ot[:, n * 512 : (n + 1) * 512],
                        V[:, kt, h * 65 : (h + 1) * 65],
                        pt[:, n * 512 : (n + 1) * 512],
                        start=(kt == 0), stop=(kt == NS - 1),
                    )

            def normalize(h, qh):
                ot = ots.pop((h, qh))
                osb = nrm_pool.tile([DK, QH], F32, tag="osb", name="osb")
                nc.vector.tensor_copy(out=osb, in_=ot[0:DK, :])
                den = nrm_pool.tile([1, QH], F32, tag="den", name="den")
                nc.vector.tensor_copy(out=den, in_=ot[64:65, :])
                recip = nrm_pool.tile([1, QH], F32, tag="recip", name="recip")
                nc.vector.reciprocal_approx_fast(recip, den)
                rbc = nrm_pool.tile([DK, QH], F32, tag="rbc", name="rbc")
                nc.gpsimd.partition_broadcast(rbc, recip)
                sl = slice(qh * QH, (qh + 1) * QH)
                dst = OC1[0:DK, sl] if h == 0 else (OC1[DK:P, sl] if h == 1 else OC2[:, sl])
                nc.vector.tensor_mul(dst, osb, rbc)

            y_r = y.rearrange("(n p) m -> n p m", p=P)

            def wo_tile(st):
                y_ps = ot_pool.tile([P, D], F32, tag="ot", name=f"y_ps{st}")
                sl = slice(st * P, (st + 1) * P)
                for n0, nn in ((0, 512), (512, 256)):
                    nc.tensor.matmul(
                        y_ps[:, n0 : n0 + nn], OC1[:, sl], wo_b1[:, n0 : n0 + nn],
                        start=True, stop=False,
                    )
                    nc.tensor.matmul(
                        y_ps[:, n0 : n0 + nn], OC2[:, sl], wo_b2[:, n0 : n0 + nn],
                        start=False, stop=True,
                    )
                y_sb = ysb_pool.tile([P, D], BF, tag="ysb", name=f"y_sb{st}")
                nc.vector.tensor_copy(out=y_sb, in_=y_ps)
                nc.sync.dma_start(out=y_r[st], in_=y_sb)

            n_steps = len(phases) * NS
            s_exp(0)
            s_exp(1)
            wo_emitted = 0
            for i in range(n_steps):
                if i + 2 < n_steps:
                    s_exp(i + 2)
                pv(i)
                h, qh = phases[i // NS]
                kt = i % NS
                if kt == NS - 1:
                    normalize(h, qh)
                    if (h, qh) == (HG - 1, 0):
                        # all heads' first q-half normalized -> first 8 wo tiles
                        for st in range(NS // 2):
                            wo_tile(st)
                            wo_emitted += 1
            for st in range(wo_emitted, NS):
                wo_tile(st)

    nc.compile()
    return nc


def kernel(query, key, value, Wq, bq, Wk, bk, Wv, bv, Wo, bo, **_ignored):
    import ml_dtypes
    from concourse.bass_utils import run_bass_kernel_spmd

    bf16 = ml_dtypes.bfloat16
    query = np.asarray(query, dtype=np.float32)
    key = np.asarray(key, dtype=np.float32)
    value = np.asarray(value, dtype=np.float32)
    Wq = np.asarray(Wq, dtype=np.float32)
    Wk = np.asarray(Wk, dtype=np.float32)
    Wv = np.asarray(Wv, dtype=np.float32)
    Wo = np.asarray(Wo, dtype=np.float32)
    bq = np.asarray(bq, dtype=np.float32)
    bk = np.asarray(bk, dtype=np.float32)
    bv = np.asarray(bv, dtype=np.float32)
    bo = np.asarray(bo, dtype=np.float32)

    use_bias_qkv = bool(np.any(bq) or np.any(bk) or np.any(bv))
    if "nc" not in _CACHE or _CACHE.get("bias") != use_bias_qkv:
        _CACHE["nc"] = _build_nc(use_bias_qkv)
        _CACHE["bias"] = use_bias_qkv
    nc = _CACHE["nc"]

    xT = {b: {} for b in range(B)}
    for b in range(B):
        xT[b]["q"] = np.ascontiguousarray(query[b].T).astype(bf16)
        xT[b]["k"] = np.ascontiguousarray(key[b].T).astype(bf16)
        xT[b]["v"] = np.ascontiguousarray(value[b].T).astype(bf16)

    in_maps = []
    for c in range(8):
        b, g = divmod(c, 4)
        hs = slice(g * HD, (g + 1) * HD)
        in_maps.append({
            "xqT": xT[b]["q"],
            "xkT": xT[b]["k"],
            "xvT": xT[b]["v"],
            "wq": np.ascontiguousarray(Wq[:, hs]).astype(bf16),
            "wk": np.ascontiguousarray(Wk[:, hs]).astype(bf16),
            "wv": np.ascontiguousarray(Wv[:, hs]).astype(bf16),
            "wo": np.ascontiguousarray(Wo[hs, :]).astype(bf16),
            "bqkv": np.ascontiguousarray(
                np.stack([bq[hs], bk[hs], bv[hs]]).astype(np.float32)
            ),
        })

    res = run_bass_kernel_spmd(nc, in_maps, core_ids=list(range(8)), **_CACHE.get("run_kwargs", {}))
    _CACHE["last_result"] = res

    out = np.empty((B, S, D), dtype=np.float32)
    for b in range(B):
        acc = res.results[4 * b]["y"].astype(np.float32)
        for g in range(1, 4):
            acc = acc + res.results[4 * b + g]["y"].astype(np.float32)
        out[b] = acc + bo[None, :]
    return out


# revision 10
# speedup vs baseline: 1.6027x; 1.0126x over previous
"""Multi-head attention (B=2, S=2048, D=768, H=12) on 8 Trainium2 cores.

Sharding: core c -> batch b = c // 4, head-group g = c % 4 (3 heads of 12).
Host prep: x^T per batch pre-transposed AND cast to bf16 (halves the x DMA
vs fp32+casting-DMA); weight shards cast to bf16.  Each core projects
Q/K/V for its 3 heads, runs attention, emits its Wo row-shard partial as
bf16; the host sums 4 partials per batch in fp32 and adds bo.

Device kernel (per core):
  - Q^T/K^T stored zero-PADDED to 128 contraction rows ([128, 3, S] tiles,
    rows 64-127 = 0) so every scores matmul is a full 128x128-array
    instruction: the 64-row (half-array / HAM k=4) config measured ~2x
    slower sustained on HW (activity throttle), and padding costs no extra
    PE cycles (row count = rhs free size).
  - All matmul outputs are <=512 fp32 columns (one PSUM bank; 1024-col out
    is an ISA violation, probed).
  - Attention runs as 6 phases (qh-major: (h0,h1,h2) x qh0 then qh1), each
    16 kt steps of: scores 2mm -> exp (ScalarE, [128,1024] tiles) -> PV
    2mm accumulating [65,1024] (ones column in V_aug rides the softmax
    denominator).  A global 2-step software pipeline (scores of step i+2
    emitted before PV of step i) keeps the in-order PE queue from ever
    waiting on the ACT exp, across phase boundaries too.
  - Wo tiles for the first q-half are emitted right after (h2,qh0)'s
    normalize, shortening the serial tail to normalize + 8 wo tiles.
"""

import sys

for _p in ("/opt/trn_rl_repo",):
    if _p not in sys.path:
        sys.path.append(_p)

import numpy as np

B = 2
S = 2048
D = 768
H = 12
DK = 64
HG = 3            # heads per core
HD = HG * DK      # 192
P = 128
NS = S // P       # 16 k-tiles
ND = D // P       # 6 d-chunks
QH = 1024         # q half

_CACHE = {}


def _build_nc(use_bias_qkv):
    import concourse.bacc as bacc
    import concourse.tile as tile
    from concourse import mybir
    from contextlib import ExitStack

    BF = mybir.dt.bfloat16
    F32 = mybir.dt.float32
    EXP = mybir.ActivationFunctionType.Exp

    nc = bacc.Bacc("TRN2", target_bir_lowering=False, debug=False)

    xqT = nc.dram_tensor("xqT", [D, S], BF, kind="ExternalInput").ap()
    xkT = nc.dram_tensor("xkT", [D, S], BF, kind="ExternalInput").ap()
    xvT = nc.dram_tensor("xvT", [D, S], BF, kind="ExternalInput").ap()
    wq = nc.dram_tensor("wq", [D, HD], BF, kind="ExternalInput").ap()
    wk = nc.dram_tensor("wk", [D, HD], BF, kind="ExternalInput").ap()
    wv = nc.dram_tensor("wv", [D, HD], BF, kind="ExternalInput").ap()
    wo = nc.dram_tensor("wo", [HD, D], BF, kind="ExternalInput").ap()
    bqkv = nc.dram_tensor("bqkv", [3, HD], F32, kind="ExternalInput").ap()
    y = nc.dram_tensor("y", [S, D], BF, kind="ExternalOutput").ap()

    with tile.TileContext(nc) as tc, ExitStack() as ctx:
        wpool = ctx.enter_context(tc.tile_pool(name="weights", bufs=1))
        apool = ctx.enter_context(tc.tile_pool(name="acts", bufs=1))

        # zero-padded transposed activations: [:, h, :] = head h, rows 64+ = 0
        KT = apool.tile([P, HG, S], BF, tag="kt")
        QT = apool.tile([P, HG, S], BF, tag="qt")
        V = apool.tile([P, NS, 3 * 65], BF, tag="v")
        OC1 = apool.tile([P, S], BF, tag="oc1")    # heads 0,1 of O^T (normalized)
        OC2 = apool.tile([P, S], BF, tag="oc2")    # head 2, rows 64-127 = 0 (keeps
                                                   # the wo matmuls in full-array config)

        # x chunk tiles (bf16 straight from HBM), all resident
        # x chunks DMA'd in s-halves, first halves of all d-chunks first, so
        # the sbp0 projections can start ~4us after the tensor's DMA begins
        xt_pool = ctx.enter_context(tc.tile_pool(name="xt", bufs=1))
        xtc = {}
        for name, xT in (("wk", xkT), ("wq", xqT), ("wv", xvT)):
            for dc in range(ND):
                xtc[(name, dc)] = xt_pool.tile(
                    [P, S], BF, tag=f"xt_{name}{dc}", name=f"xt_{name}{dc}"
                )
            for half in range(2):
                hsl = slice(half * QH, (half + 1) * QH)
                for dc in range(ND):
                    nc.gpsimd.dma_start(
                        out=xtc[(name, dc)][:, hsl],
                        in_=xT[dc * P : (dc + 1) * P, hsl],
                    )

        # weights (bf16 on host, no device casts), HWDGE queue
        w_bf = {}
        for name, w in (("wk", wk), ("wq", wq), ("wv", wv)):
            wb = wpool.tile([P, ND, HD], BF, tag=f"{name}_bf", name=f"{name}_bf")
            nc.sync.dma_start(out=wb, in_=w.rearrange("(nd p) h -> p nd h", p=P))
            w_bf[name] = wb
        wo_b1 = wpool.tile([P, D], BF, tag="wo_b1")
        nc.sync.dma_start(out=wo_b1, in_=wo[0:P, :])
        wo_b2 = wpool.tile([P, D], BF, tag="wo_b2")   # rows 64-127 = 0 (padding)
        nc.sync.dma_start(out=wo_b2[0:DK, :], in_=wo[P:HD, :])

        bias_a = {}
        bias_b = {}
        bias_vrow = None
        if use_bias_qkv:
            for i, name in enumerate(("wq", "wk", "wv")):
                ba = wpool.tile([P, 1], F32, tag=f"ba_{name}", name=f"ba_{name}")
                nc.sync.dma_start(out=ba, in_=bqkv[i, 0:P].rearrange("p -> p 1"))
                bb = wpool.tile([DK, 1], F32, tag=f"bb_{name}", name=f"bb_{name}")
                nc.sync.dma_start(out=bb, in_=bqkv[i, P:HD].rearrange("p -> p 1"))
                bias_a[name] = ba
                bias_b[name] = bb
            # V bias varies along the free dim of psV [s, 192]: broadcast the
            # bias row across all 128 partitions once
            bvr = wpool.tile([1, HD], F32, tag="bv_row")
            nc.sync.dma_start(out=bvr, in_=bqkv[2, :].rearrange("h -> 1 h"))
            bias_vrow = wpool.tile([P, HD], F32, tag="bv_bcast")
            nc.gpsimd.partition_broadcast(bias_vrow, bvr)

        # padding zeros + V ones columns (off the PE path; after DMA triggers)
        nc.gpsimd.memset(KT[DK:P, :, :], 0.0)
        nc.vector.memset(QT[DK:P, :, :], 0.0)
        nc.vector.memset(V[:, :, 64 : 3 * 65 : 65], 1.0)
        nc.gpsimd.memset(OC2[DK:P, :], 0.0)
        nc.vector.memset(wo_b2[DK:P, :], 0.0)

        # ================= phase 1: projections =================
        with tc.tile_pool(name="ppa", bufs=2, space="PSUM") as ppa_pool, \
             tc.tile_pool(name="ppb", bufs=1, space="PSUM") as ppb_pool, \
             tc.tile_pool(name="psv", bufs=2, space="PSUM") as psv_pool:

            def qk_proj(name, dst):
                wb = w_bf[name]
                for sbp in range(2):
                    sl = slice(sbp * QH, (sbp + 1) * QH)
                    psA = ppa_pool.tile([P, QH], F32, tag="ppa", name=f"psA_{name}{sbp}")
                    psB = ppb_pool.tile([DK, QH], F32, tag="ppb", name=f"psB_{name}{sbp}")
                    for d in range(ND):
                        xt_d = xtc[(name, d)]
                        for half in range(2):
                            hsl = slice(half * 512, (half + 1) * 512)
                            xsl = slice(sbp * QH + half * 512, sbp * QH + (half + 1) * 512)
                            nc.tensor.matmul(
                                psA[:, hsl], wb[:, d, 0:P], xt_d[:, xsl],
                                start=(d == 0), stop=(d == ND - 1),
                            )
                    for d in range(ND):
                        xt_d = xtc[(name, d)]
                        for half in range(2):
                            hsl = slice(half * 512, (half + 1) * 512)
                            xsl = slice(sbp * QH + half * 512, sbp * QH + (half + 1) * 512)
                            nc.tensor.matmul(
                                psB[:, hsl], wb[:, d, P:HD], xt_d[:, xsl],
                                start=(d == 0), stop=(d == ND - 1),
                            )
                    if use_bias_qkv:
                        ba, bb = bias_a[name], bias_b[name]
                        nc.vector.tensor_scalar_add(dst[0:DK, 0, sl], psA[0:DK, :], ba[0:DK])
                        nc.vector.tensor_scalar_add(dst[0:DK, 1, sl], psA[DK:P, :], ba[DK:P])
                        nc.vector.tensor_scalar_add(dst[0:DK, 2, sl], psB, bb)
                    else:
                        nc.vector.tensor_copy(out=dst[0:DK, 0, sl], in_=psA[0:DK, :])
                        nc.vector.tensor_copy(out=dst[0:DK, 1, sl], in_=psA[DK:P, :])
                        nc.vector.tensor_copy(out=dst[0:DK, 2, sl], in_=psB)

            qk_proj("wk", KT)
            qk_proj("wq", QT)

            wb = w_bf["wv"]
            for st in range(NS):
                psV = psv_pool.tile([P, HD], F32, tag="psv", name=f"psV{st}")
                for d in range(ND):
                    nc.tensor.matmul(
                        psV, xtc[("wv", d)][:, st * P : (st + 1) * P], wb[:, d, :],
                        start=(d == 0), stop=(d == ND - 1),
                    )
                for h in range(HG):
                    if use_bias_qkv:
                        nc.vector.tensor_add(
                            V[:, st, h * 65 : h * 65 + 64],
                            psV[:, h * DK : (h + 1) * DK],
                            bias_vrow[:, h * DK : (h + 1) * DK],
                        )
                    else:
                        nc.vector.tensor_copy(
                            out=V[:, st, h * 65 : h * 65 + 64],
                            in_=psV[:, h * DK : (h + 1) * DK],
                        )

        # ============ phase 2: attention (+ wo) ============
        with tc.tile_pool(name="s_ps", bufs=2, space="PSUM") as s_pool, \
             tc.tile_pool(name="ot_ps", bufs=2, space="PSUM") as ot_pool, \
             tc.tile_pool(name="pt", bufs=4) as pt_pool, \
             tc.tile_pool(name="nrm", bufs=2) as nrm_pool, \
             tc.tile_pool(name="y_sb", bufs=2) as ysb_pool:

            # qh-major so wo for q-half 0 can run before the final phase
            phases = [(h, 0) for h in range(HG)] + [(h, 1) for h in range(HG)]
            ots = {}
            pts = {}

            def s_exp(i):
                h, qh = phases[i // NS]
                kt = i % NS
                s_ps = s_pool.tile([P, QH], F32, tag="s", name=f"s{i}")
                for n in range(2):
                    q0 = qh * QH + n * 512
                    nc.tensor.matmul(
                        s_ps[:, n * 512 : (n + 1) * 512],
                        KT[:, h, kt * P : (kt + 1) * P],
                        QT[:, h, q0 : q0 + 512],
                        start=True, stop=True,
                    )
                pt = pt_pool.tile([P, QH], BF, tag="pt", name=f"pt{i}")
                nc.scalar.activation(pt, s_ps, EXP, bias=0.0, scale=0.125)
                pts[i] = pt

            def pv(i):
                h, qh = phases[i // NS]
                kt = i % NS
                if kt == 0:
                    ots[(h, qh)] = ot_pool.tile([65, QH], F32, tag="ot", name=f"ot{h}_{qh}")
                ot = ots[(h, qh)]
                pt = pts.pop(i)
                for n in range(2):
                    nc.tensor.matmul(
                        ot[:, n * 512 : (n + 1) * 512],
                        V[:, kt, h * 65 : (h + 1) * 65],
                        pt[:, n * 512 : (n + 1) * 512],
                        start=(kt == 0), stop=(kt == NS - 1),
                    )

            def normalize(h, qh):
                ot = ots.pop((h, qh))
                osb = nrm_pool.tile([DK, QH], F32, tag="osb", name="osb")
                nc.vector.tensor_copy(out=osb, in_=ot[0:DK, :])
                den = nrm_pool.tile([1, QH], F32, tag="den", name="den")
                nc.vector.tensor_copy(out=den, in_=ot[64:65, :])
                recip = nrm_pool.tile([1, QH], F32, tag="recip", name="recip")
                nc.vector.reciprocal_approx_fast(recip, den)
                rbc = nrm_pool.tile([DK, QH], F32, tag="rbc", name="rbc")
                nc.gpsimd.partition_broadcast(rbc, recip)
                sl = slice(qh * QH, (qh + 1) * QH)
                dst = OC1[0:DK, sl] if h == 0 else (OC1[DK:P, sl] if h == 1 else OC2[0:DK, sl])
                nc.vector.tensor_mul(dst, osb, rbc)

            y_r = y.rearrange("(n p) m -> n p m", p=P)

            def wo_tile(st):
                y_ps = ot_pool.tile([P, D], F32, tag="ot", name=f"y_ps{st}")
                sl = slice(st * P, (st + 1) * P)
                for n0, nn in ((0, 512), (512, 256)):
                    nc.tensor.matmul(
                        y_ps[:, n0 : n0 + nn], OC1[:, sl], wo_b1[:, n0 : n0 + nn],
                        start=True, stop=False,
                    )
                    nc.tensor.matmul(
                        y_ps[:, n0 : n0 + nn], OC2[:, sl], wo_b2[:, n0 : n0 + nn],
                        start=False, stop=True,
                    )
                y_sb = ysb_pool.tile([P, D], BF, tag="ysb", name=f"y_sb{st}")
                nc.vector.tensor_copy(out=y_sb, in_=y_ps)
                nc.sync.dma_start(out=y_r[st], in_=y_sb)

            n_steps = len(phases) * NS
            s_exp(0)
            s_exp(1)
            wo_emitted = 0
            for i in range(n_steps):
                if i + 2 < n_steps:
                    s_exp(i + 2)
                pv(i)
                h, qh = phases[i // NS]
                kt = i % NS
                if kt == NS - 1:
                    normalize(h, qh)
                    if (h, qh) == (HG - 1, 0):
                        # all heads' first q-half normalized -> first 8 wo tiles
                        for st in range(NS // 2):
                            wo_tile(st)
                            wo_emitted += 1
            for st in range(wo_emitted, NS):
                wo_tile(st)

    nc.compile()
    return nc


def kernel(query, key, value, Wq, bq, Wk, bk, Wv, bv, Wo, bo, **_ignored):
    import ml_dtypes
    from concourse.bass_utils import run_bass_kernel_spmd

    bf16 = ml_dtypes.bfloat16
    query = np.asarray(query, dtype=np.float32)
    key = np.asarray(key, dtype=np.float32)
    value = np.asarray(value, dtype=np.float32)
    Wq = np.asarray(Wq, dtype=np.float32)
    Wk = np.asarray(Wk, dtype=np.float32)
    Wv = np.asarray(Wv, dtype=np.float32)
    Wo = np.asarray(Wo, dtype=np.float32)
    bq = np.asarray(bq, dtype=np.float32)
    bk = np.asarray(bk, dtype=np.float32)
    bv = np.asarray(bv, dtype=np.float32)
    bo = np.asarray(bo, dtype=np.float32)

    use_bias_qkv = bool(np.any(bq) or np.any(bk) or np.any(bv))
    if "nc" not in _CACHE or _CACHE.get("bias") != use_bias_qkv:
        _CACHE["nc"] = _build_nc(use_bias_qkv)
        _CACHE["bias"] = use_bias_qkv
    nc = _CACHE["nc"]

    xT = {b: {} for b in range(B)}
    for b in range(B):
        xT[b]["q"] = np.ascontiguousarray(query[b].T).astype(bf16)
        xT[b]["k"] = np.ascontiguousarray(key[b].T).astype(bf16)
        xT[b]["v"] = np.ascontiguousarray(value[b].T).astype(bf16)

    in_maps = []
    for c in range(8):
        b, g = divmod(c, 4)
        hs = slice(g * HD, (g + 1) * HD)
        in_maps.append({
            "xqT": xT[b]["q"],
            "xkT": xT[b]["k"],
            "xvT": xT[b]["v"],
            "wq": np.ascontiguousarray(Wq[:, hs]).astype(bf16),
            "wk": np.ascontiguousarray(Wk[:, hs]).astype(bf16),
            "wv": np.ascontiguousarray(Wv[:, hs]).astype(bf16),
            "wo": np.ascontiguousarray(Wo[hs, :]).astype(bf16),
            "bqkv": np.ascontiguousarray(
                np.stack([bq[hs], bk[hs], bv[hs]]).astype(np.float32)
            ),
        })

    res = run_bass_kernel_spmd(nc, in_maps, core_ids=list(range(8)), **_CACHE.get("run_kwargs", {}))
    _CACHE["last_result"] = res

    out = np.empty((B, S, D), dtype=np.float32)
    for b in range(B):
        acc = res.results[4 * b]["y"].astype(np.float32)
        for g in range(1, 4):
            acc = acc + res.results[4 * b + g]["y"].astype(np.float32)
        out[b] = acc + bo[None, :]
    return out


# revision 14
# speedup vs baseline: 1.6701x; 1.0421x over previous
"""Multi-head attention (B=2, S=2048, D=768, H=12) on 8 Trainium2 cores.

Sharding: core c -> batch b = c // 4, head-group g = c % 4 (3 heads of 12).
Host prep: x^T per batch pre-transposed AND cast to bf16 (halves the x DMA
vs fp32+casting-DMA); weight shards cast to bf16.  Each core projects
Q/K/V for its 3 heads, runs attention, emits its Wo row-shard partial as
bf16; the host sums 4 partials per batch in fp32 and adds bo.

Device kernel (per core):
  - Q^T/K^T stored zero-PADDED to 128 contraction rows ([128, 3, S] tiles,
    rows 64-127 = 0) so every scores matmul is a full 128x128-array
    instruction: the 64-row (half-array / HAM k=4) config measured ~2x
    slower sustained on HW (activity throttle), and padding costs no extra
    PE cycles (row count = rhs free size).
  - All matmul outputs are <=512 fp32 columns (one PSUM bank; 1024-col out
    is an ISA violation, probed).
  - Attention runs as 6 phases (qh-major: (h0,h1,h2) x qh0 then qh1), each
    16 kt steps of: scores 2mm -> exp (ScalarE, [128,1024] tiles) -> PV
    2mm accumulating [65,1024] (ones column in V_aug rides the softmax
    denominator).  A global 2-step software pipeline (scores of step i+2
    emitted before PV of step i) keeps the in-order PE queue from ever
    waiting on the ACT exp, across phase boundaries too.
  - Wo tiles for the first q-half are emitted right after (h2,qh0)'s
    normalize, shortening the serial tail to normalize + 8 wo tiles.
"""

import sys

for _p in ("/opt/trn_rl_repo",):
    if _p not in sys.path:
        sys.path.append(_p)

import numpy as np

B = 2
S = 2048
D = 768
H = 12
DK = 64
HG = 3            # heads per core
HD = HG * DK      # 192
P = 128
NS = S // P       # 16 k-tiles
ND = D // P       # 6 d-chunks
QH = 1024         # q half

_CACHE = {}


def _build_nc(use_bias_qkv):
    import concourse.bacc as bacc
    import concourse.tile as tile
    from concourse import mybir
    from contextlib import ExitStack

    BF = mybir.dt.bfloat16
    F32 = mybir.dt.float32
    EXP = mybir.ActivationFunctionType.Exp

    nc = bacc.Bacc("TRN2", target_bir_lowering=False, debug=False)

    xqT = nc.dram_tensor("xqT", [D, S], BF, kind="ExternalInput").ap()
    xkT = nc.dram_tensor("xkT", [D, S], BF, kind="ExternalInput").ap()
    xvT = nc.dram_tensor("xvT", [D, S], BF, kind="ExternalInput").ap()
    wq = nc.dram_tensor("wq", [D, HD], BF, kind="ExternalInput").ap()
    wk = nc.dram_tensor("wk", [D, HD], BF, kind="ExternalInput").ap()
    wv = nc.dram_tensor("wv", [D, HD], BF, kind="ExternalInput").ap()
    wo = nc.dram_tensor("wo", [HD, D], BF, kind="ExternalInput").ap()
    bqkv = nc.dram_tensor("bqkv", [3, HD], F32, kind="ExternalInput").ap()
    y = nc.dram_tensor("y", [S, D], BF, kind="ExternalOutput").ap()

    with tile.TileContext(nc) as tc, ExitStack() as ctx:
        wpool = ctx.enter_context(tc.tile_pool(name="weights", bufs=1))
        apool = ctx.enter_context(tc.tile_pool(name="acts", bufs=1))

        # zero-padded transposed activations: [:, h, :] = head h, rows 64+ = 0
        KT = apool.tile([P, HG, S], BF, tag="kt")
        QT = apool.tile([P, HG, S], BF, tag="qt")
        V = apool.tile([P, NS, 3 * 65], BF, tag="v")
        OC1 = apool.tile([P, S], BF, tag="oc1")    # heads 0,1 of O^T (normalized)
        OC2 = apool.tile([P, S], BF, tag="oc2")    # head 2, rows 64-127 = 0 (keeps
                                                   # the wo matmuls in full-array config)

        # x chunk tiles (bf16 straight from HBM), all resident
        # x chunks DMA'd in s-halves, first halves of all d-chunks first, so
        # the sbp0 projections can start ~4us after the tensor's DMA begins
        xt_pool = ctx.enter_context(tc.tile_pool(name="xt", bufs=1))
        xtc = {}
        for name, xT in (("wk", xkT), ("wq", xqT), ("wv", xvT)):
            for dc in range(ND):
                xtc[(name, dc)] = xt_pool.tile(
                    [P, S], BF, tag=f"xt_{name}{dc}", name=f"xt_{name}{dc}"
                )
            for half in range(2):
                hsl = slice(half * QH, (half + 1) * QH)
                for dc in range(ND):
                    nc.gpsimd.dma_start(
                        out=xtc[(name, dc)][:, hsl],
                        in_=xT[dc * P : (dc + 1) * P, hsl],
                    )

        # weights (bf16 on host, no device casts), HWDGE queue
        w_bf = {}
        for name, w in (("wk", wk), ("wq", wq), ("wv", wv)):
            wb = wpool.tile([P, ND, HD], BF, tag=f"{name}_bf", name=f"{name}_bf")
            nc.sync.dma_start(out=wb, in_=w.rearrange("(nd p) h -> p nd h", p=P))
            w_bf[name] = wb
        wo_b1 = wpool.tile([P, D], BF, tag="wo_b1")
        nc.sync.dma_start(out=wo_b1, in_=wo[0:P, :])
        wo_b2 = wpool.tile([P, D], BF, tag="wo_b2")   # rows 64-127 = 0 (padding)
        nc.sync.dma_start(out=wo_b2[0:DK, :], in_=wo[P:HD, :])

        bias_a = {}
        bias_b = {}
        bias_vrow = None
        if use_bias_qkv:
            for i, name in enumerate(("wq", "wk", "wv")):
                ba = wpool.tile([P, 1], F32, tag=f"ba_{name}", name=f"ba_{name}")
                nc.sync.dma_start(out=ba, in_=bqkv[i, 0:P].rearrange("p -> p 1"))
                bb = wpool.tile([DK, 1], F32, tag=f"bb_{name}", name=f"bb_{name}")
                nc.sync.dma_start(out=bb, in_=bqkv[i, P:HD].rearrange("p -> p 1"))
                bias_a[name] = ba
                bias_b[name] = bb
            # V bias varies along the free dim of psV [s, 192]: broadcast the
            # bias row across all 128 partitions once
            bvr = wpool.tile([1, HD], F32, tag="bv_row")
            nc.sync.dma_start(out=bvr, in_=bqkv[2, :].rearrange("h -> 1 h"))
            bias_vrow = wpool.tile([P, HD], F32, tag="bv_bcast")
            nc.gpsimd.partition_broadcast(bias_vrow, bvr)

        # padding zeros + V ones columns (off the PE path; after DMA triggers)
        nc.gpsimd.memset(KT[DK:P, :, :], 0.0)
        nc.vector.memset(QT[DK:P, :, :], 0.0)
        nc.vector.memset(V[:, :, 64 : 3 * 65 : 65], 1.0)
        nc.gpsimd.memset(OC2[DK:P, :], 0.0)
        nc.vector.memset(wo_b2[DK:P, :], 0.0)

        # ================= phase 1: projections =================
        with tc.tile_pool(name="ppa", bufs=2, space="PSUM") as ppa_pool, \
             tc.tile_pool(name="ppb", bufs=1, space="PSUM") as ppb_pool, \
             tc.tile_pool(name="psv", bufs=2, space="PSUM") as psv_pool:

            def qk_proj(name, dst):
                wb = w_bf[name]
                for sbp in range(2):
                    sl = slice(sbp * QH, (sbp + 1) * QH)
                    psA = ppa_pool.tile([P, QH], F32, tag="ppa", name=f"psA_{name}{sbp}")
                    psB = ppb_pool.tile([DK, QH], F32, tag="ppb", name=f"psB_{name}{sbp}")
                    for d in range(ND):
                        xt_d = xtc[(name, d)]
                        for half in range(2):
                            hsl = slice(half * 512, (half + 1) * 512)
                            xsl = slice(sbp * QH + half * 512, sbp * QH + (half + 1) * 512)
                            nc.tensor.matmul(
                                psA[:, hsl], wb[:, d, 0:P], xt_d[:, xsl],
                                start=(d == 0), stop=(d == ND - 1),
                            )
                    for d in range(ND):
                        xt_d = xtc[(name, d)]
                        for half in range(2):
                            hsl = slice(half * 512, (half + 1) * 512)
                            xsl = slice(sbp * QH + half * 512, sbp * QH + (half + 1) * 512)
                            nc.tensor.matmul(
                                psB[:, hsl], wb[:, d, P:HD], xt_d[:, xsl],
                                start=(d == 0), stop=(d == ND - 1),
                            )
                    if use_bias_qkv:
                        ba, bb = bias_a[name], bias_b[name]
                        nc.vector.tensor_scalar_add(dst[0:DK, 0, sl], psA[0:DK, :], ba[0:DK])
                        nc.vector.tensor_scalar_add(dst[0:DK, 1, sl], psA[DK:P, :], ba[DK:P])
                        nc.vector.tensor_scalar_add(dst[0:DK, 2, sl], psB, bb)
                    else:
                        nc.vector.tensor_copy(out=dst[0:DK, 0, sl], in_=psA[0:DK, :])
                        nc.vector.tensor_copy(out=dst[0:DK, 1, sl], in_=psA[DK:P, :])
                        nc.vector.tensor_copy(out=dst[0:DK, 2, sl], in_=psB)

            qk_proj("wk", KT)
            qk_proj("wq", QT)

            wb = w_bf["wv"]
            for st in range(NS):
                psV = psv_pool.tile([P, HD], F32, tag="psv", name=f"psV{st}")
                for d in range(ND):
                    nc.tensor.matmul(
                        psV, xtc[("wv", d)][:, st * P : (st + 1) * P], wb[:, d, :],
                        start=(d == 0), stop=(d == ND - 1),
                    )
                for h in range(HG):
                    if use_bias_qkv:
                        nc.vector.tensor_add(
                            V[:, st, h * 65 : h * 65 + 64],
                            psV[:, h * DK : (h + 1) * DK],
                            bias_vrow[:, h * DK : (h + 1) * DK],
                        )
                    else:
                        nc.vector.tensor_copy(
                            out=V[:, st, h * 65 : h * 65 + 64],
                            in_=psV[:, h * DK : (h + 1) * DK],
                        )

        # ============ phase 2: attention (+ wo) ============
        with tc.tile_pool(name="s_ps", bufs=2, space="PSUM") as s_pool, \
             tc.tile_pool(name="ot_ps", bufs=2, space="PSUM") as ot_pool, \
             tc.tile_pool(name="pt", bufs=4) as pt_pool, \
             tc.tile_pool(name="nrm", bufs=2) as nrm_pool, \
             tc.tile_pool(name="y_sb", bufs=2) as ysb_pool:

            # qh-major so wo for q-half 0 can run before the final phase
            phases = [(h, 0) for h in range(HG)] + [(h, 1) for h in range(HG)]
            ots = {}
            pts = {}

            def s_exp(i):
                h, qh = phases[i // NS]
                kt = i % NS
                s_ps = s_pool.tile([P, QH], F32, tag="s", name=f"s{i}")
                for n in range(2):
                    q0 = qh * QH + n * 512
                    nc.tensor.matmul(
                        s_ps[:, n * 512 : (n + 1) * 512],
                        KT[:, h, kt * P : (kt + 1) * P],
                        QT[:, h, q0 : q0 + 512],
                        start=True, stop=True,
                    )
                pt = pt_pool.tile([P, QH], BF, tag="pt", name=f"pt{i}")
                nc.scalar.activation(pt, s_ps, EXP, bias=0.0, scale=0.125)
                pts[i] = pt

            def pv(i):
                h, qh = phases[i // NS]
                kt = i % NS
                if kt == 0:
                    ots[(h, qh)] = ot_pool.tile([65, QH], F32, tag="ot", name=f"ot{h}_{qh}")
                ot = ots[(h, qh)]
                pt = pts.pop(i)
                for n in range(2):
                    nc.tensor.matmul(
                        ot[:, n * 512 : (n + 1) * 512],
                        V[:, kt, h * 65 : (h + 1) * 65],
                        pt[:, n * 512 : (n + 1) * 512],
                        start=(kt == 0), stop=(kt == NS - 1),
                    )

            def normalize(h, qh):
                ot = ots.pop((h, qh))
                osb = nrm_pool.tile([DK, QH], F32, tag="osb", name="osb")
                nc.vector.tensor_copy(out=osb, in_=ot[0:DK, :])
                den = nrm_pool.tile([1, QH], F32, tag="den", name="den")
                nc.scalar.copy(den, ot[64:65, :])
                recip = nrm_pool.tile([1, QH], F32, tag="recip", name="recip")
                nc.vector.reciprocal_approx_fast(recip, den)
                rbc = nrm_pool.tile([DK, QH], F32, tag="rbc", name="rbc")
                nc.gpsimd.partition_broadcast(rbc, recip)
                sl = slice(qh * QH, (qh + 1) * QH)
                dst = OC1[0:DK, sl] if h == 0 else (OC1[DK:P, sl] if h == 1 else OC2[0:DK, sl])
                nc.vector.tensor_mul(dst, osb, rbc)

            y_r = y.rearrange("(n p) m -> n p m", p=P)

            def wo_tile(st):
                y_ps = ot_pool.tile([P, D], F32, tag="ot", name=f"y_ps{st}")
                sl = slice(st * P, (st + 1) * P)
                # region-major order: adjacent matmuls never accumulate into
                # the same PSUM region (back-to-back same-bank accumulation
                # serializes the PE)
                for oc, wob, start, stop in (
                    (OC1, wo_b1, True, False),
                    (OC2, wo_b2, False, True),
                ):
                    for n0, nn in ((0, 512), (512, 256)):
                        nc.tensor.matmul(
                            y_ps[:, n0 : n0 + nn], oc[:, sl], wob[:, n0 : n0 + nn],
                            start=start, stop=stop,
                        )
                y_sb = ysb_pool.tile([P, D], BF, tag="ysb", name=f"y_sb{st}")
                nc.vector.tensor_copy(out=y_sb, in_=y_ps)
                nc.sync.dma_start(out=y_r[st], in_=y_sb)

            n_steps = len(phases) * NS
            s_exp(0)
            s_exp(1)
            wo_pending = []
            wo_emitted = 0
            for i in range(n_steps):
                if i + 2 < n_steps:
                    s_exp(i + 2)
                pv(i)
                if wo_pending:
                    # spread q-half-0 wo tiles one per step through the
                    # qh1 phases instead of a blocking burst
                    wo_tile(wo_pending.pop(0))
                    wo_emitted += 1
                h, qh = phases[i // NS]
                kt = i % NS
                if kt == NS - 1:
                    normalize(h, qh)
                    if (h, qh) == (HG - 1, 0):
                        wo_pending = list(range(NS // 2))
            for st in range(wo_emitted, NS):
                wo_tile(st)

    nc.compile()
    return nc


def kernel(query, key, value, Wq, bq, Wk, bk, Wv, bv, Wo, bo, **_ignored):
    import ml_dtypes
    from concourse.bass_utils import run_bass_kernel_spmd

    bf16 = ml_dtypes.bfloat16
    query = np.asarray(query, dtype=np.float32)
    key = np.asarray(key, dtype=np.float32)
    value = np.asarray(value, dtype=np.float32)
    Wq = np.asarray(Wq, dtype=np.float32)
    Wk = np.asarray(Wk, dtype=np.float32)
    Wv = np.asarray(Wv, dtype=np.float32)
    Wo = np.asarray(Wo, dtype=np.float32)
    bq = np.asarray(bq, dtype=np.float32)
    bk = np.asarray(bk, dtype=np.float32)
    bv = np.asarray(bv, dtype=np.float32)
    bo = np.asarray(bo, dtype=np.float32)

    use_bias_qkv = bool(np.any(bq) or np.any(bk) or np.any(bv))
    if "nc" not in _CACHE or _CACHE.get("bias") != use_bias_qkv:
        _CACHE["nc"] = _build_nc(use_bias_qkv)
        _CACHE["bias"] = use_bias_qkv
    nc = _CACHE["nc"]

    xT = {b: {} for b in range(B)}
    for b in range(B):
        xT[b]["q"] = np.ascontiguousarray(query[b].T).astype(bf16)
        xT[b]["k"] = np.ascontiguousarray(key[b].T).astype(bf16)
        xT[b]["v"] = np.ascontiguousarray(value[b].T).astype(bf16)

    in_maps = []
    for c in range(8):
        b, g = divmod(c, 4)
        hs = slice(g * HD, (g + 1) * HD)
        in_maps.append({
            "xqT": xT[b]["q"],
            "xkT": xT[b]["k"],
            "xvT": xT[b]["v"],
            "wq": np.ascontiguousarray(Wq[:, hs]).astype(bf16),
            "wk": np.ascontiguousarray(Wk[:, hs]).astype(bf16),
            "wv": np.ascontiguousarray(Wv[:, hs]).astype(bf16),
            "wo": np.ascontiguousarray(Wo[hs, :]).astype(bf16),
            "bqkv": np.ascontiguousarray(
                np.stack([bq[hs], bk[hs], bv[hs]]).astype(np.float32)
            ),
        })

    res = run_bass_kernel_spmd(nc, in_maps, core_ids=list(range(8)), **_CACHE.get("run_kwargs", {}))
    _CACHE["last_result"] = res

    out = np.empty((B, S, D), dtype=np.float32)
    for b in range(B):
        acc = res.results[4 * b]["y"].astype(np.float32)
        for g in range(1, 4):
            acc = acc + res.results[4 * b + g]["y"].astype(np.float32)
        out[b] = acc + bo[None, :]
    return out
